# revision 135
# baseline (speedup 1.0000x reference)
"""Causal multi-head attention on 8 Trainium2 NeuronCores (Bass/Tile).

Problem (hardcoded): x[2,2048,1024], W_qkv[1024,3072], b_qkv[3072],
W_proj[1024,1024], b_proj[1024]; 16 heads, head_dim 64, causal softmax.

Sharding: tensor-parallel over heads — core c owns heads (2c, 2c+1).
Each core computes qkv for its 2 heads (needs full x), the causal
attention for those heads, and a row-parallel partial of the output
projection. Host sums the 8 partials and adds the (precomputable) bias
terms.

Device layout choices (all chosen to avoid on-device transposes):
  - x is passed host-transposed as xT[1024, 4096] so the PE (which
    contracts over the partition dim) can consume it directly.
  - q,k are produced transposed (qT/kT [128=2*64, 4096]) straight out of
    the qkv matmul; v is produced in natural [token, feat] layout via a
    PE transpose of the vT matmul result.
  - attention scores are computed as S^T = k @ q^T in [tk, tq] blocks of
    [128, 2*512] (both heads side by side); causal masking is a 0/1
    multiply on VectorE for the in-block triangle only; fully-masked
    column ranges of diagonal superblocks are skipped outright in the
    S matmul, the exp, and the P@V matmul (partial-width PSUM
    accumulation; stop only on the bank's final matmul).
  - P@V uses V widened with a 64-column ones block, so the softmax
    denominator lands bank-aligned under the numerator in the same PSUM
    tile; normalization is an elementwise reciprocal+multiply.
  - emission is software-pipelined (A(n) / B(b,i) / C slices
    interleaved) so the PE-heavy qkv/proj phases overlap the
    ScalarE-heavy exp phase.
All matmul/DMA data tensors are bf16 (PSUM accumulation stays fp32):
same PE rate as f32r for wide outputs, but no 4x penalty for outputs
narrower than 256 (needed for causal skipping) and half the DMA bytes.
"""

import numpy as np
import ml_dtypes

import concourse.bass as bass
import concourse.tile as tile
from concourse import bacc, mybir
from concourse.bass_utils import run_bass_kernel_spmd

B, T, C = 2, 2048, 1024
H, D = 16, 64
TOK = B * T            # 4096
P = 128
NQ = 512               # q-chunk (moving free dim per head)
KB = 128               # k-block (PSUM partition dim)
KO = C // P            # 8 contraction subtiles
NCHUNK = TOK // NQ     # 8 token chunks
QC = T // NQ           # 4 q-chunks per batch
KBB = T // KB          # 16 k-blocks per batch
F32 = mybir.dt.float32
BF16 = mybir.dt.bfloat16
BFNP = ml_dtypes.bfloat16

_CACHE = {}


def _build():
    nc = bacc.Bacc("TRN2", target_bir_lowering=False, debug=False, num_devices=8)
    marks = []
    _CACHE["marks"] = marks

    def mark(lbl):
        marks.append((nc.next_id(), lbl))

    xt_d = nc.dram_tensor("xt", [C, TOK], BF16, kind="ExternalInput").ap()
    wqk_d = nc.dram_tensor("wqk", [C, 256], BF16, kind="ExternalInput").ap()
    bqk_d = nc.dram_tensor("bqk", [P, 2], F32, kind="ExternalInput").ap()
    wv_d = nc.dram_tensor("wv", [C, P], BF16, kind="ExternalInput").ap()
    wproj_d = nc.dram_tensor("wproj", [P, C], BF16, kind="ExternalInput").ap()
    masks_d = nc.dram_tensor("masks", [P, P], BF16, kind="ExternalInput").ap()
    ident_d = nc.dram_tensor("ident", [P, P], BF16, kind="ExternalInput").ap()
    y_d = nc.dram_tensor("y", [TOK, C], BF16, kind="ExternalOutput").ap()

    wqk_dr = wqk_d.rearrange("(ko p) m -> p ko m", p=P)
    xt_dr = xt_d.rearrange("(ko p) m -> p ko m", p=P)

    with tile.TileContext(nc) as tc:
        with tc.tile_pool(name="res", bufs=1) as res, \
             tc.tile_pool(name="xt", bufs=8) as xtp, \
             tc.tile_pool(name="pt", bufs=8) as ptp, \
             tc.tile_pool(name="ystage", bufs=12) as ysp:
            # ---- resident tensors ----
            wqk_sb = res.tile([P, KO, 256], BF16, tag="wqk")
            bqk_sb = res.tile([P, 2], F32, tag="bqk")
            wv_sb = res.tile([P, KO, P], BF16, tag="wv")
            wproj_sb = res.tile([P, C], BF16, tag="wproj")
            masks_sb = res.tile([P, P], BF16, tag="masks")
            ident_sb = res.tile([P, P], BF16, tag="ident")

            qT_sb = res.tile([P, TOK], BF16, tag="qT")
            kT_sb = res.tile([P, TOK], BF16, tag="kT")
            v_sb = [res.tile([P, 2 * KBB, 2 * D], BF16, tag=f"v{h}", name=f"v{h}")
                    for h in range(2)]
            attns_sb = res.tile([P, TOK], BF16, tag="attns")

            # PE warm-up scratch goes first on GPSIMD (its queue is empty
            # at t=0); the v ones blocks (denominator replicator) are
            # emitted after the resident SWDGE loads in the prologue
            warm_sb = res.tile([P, NQ], BF16, tag="warm")
            nc.gpsimd.memset(warm_sb[:], 0.0)



            # ---- filler machinery: A(qkv) and C(proj) work is split into
            # small PE quanta pumped between attention j-steps, so the PE
            # (in-order queue) always has ready work while ScalarE runs exp.
            # A quanta pop first; C quanta are rationed (their 2 shared PSUM
            # banks recycle through a DVE copy, so back-to-back C quanta
            # stall the in-order PE queue) and a couple are reserved for the
            # drain so its first matmuls read long-normalized attns slices.
            from collections import deque
            fillA = deque()
            fillC = deque()          # entries: (birth_step, closure)
            RESERVE_C = 1
            step_box = [0]           # j-steps emitted so far
            # C quanta: (pool, tag-prefix); at the drain the quanta rotate
            # over the freed attention PSUM banks and their copies alternate
            # DVE/Act (Act is idle then).
            cpool_box = [None]
            xt0_box = [None]
            drain_box = [False]
            alt_box = [False]   # alternate C copies DVE/Act (Act has slack)
            ccount_box = [0]

            def pump(k=1, max_c=None):
                n = 0
                n_c = 0
                while n < k:
                    if fillA:
                        f = fillA.popleft()
                    elif fillC and (drain_box[0]
                                    or (len(fillC) > RESERVE_C
                                        and (max_c is None or n_c < max_c)
                                        # cooldown: a fresh C quantum's attns
                                        # normalization is still queued on
                                        # DVE; pumping it would stall PE
                                        and step_box[0] >= fillC[0][0] + 2)):
                        f = fillC.popleft()[1]
                        n_c += 1
                    else:
                        break
                    f()
                    n += 1

            def make_A_quanta(n, psF):
                st = {}

                def q_dma():
                    if n == 0:
                        # chunk-0 DMAs were hand-ordered in the prologue
                        st["get"] = lambda k: xt0_box[0][:, k, :]
                        return
                    xt = xtp.tile([P, KO, NQ], BF16, name="xtc", tag="xtc",
                                  bufs=4)
                    nc.sync.dma_start(
                        xt[:], xt_dr[:, :, n * NQ:(n + 1) * NQ])
                    st["get"] = lambda k: xt[:, k, :]

                def q_qk(m):
                    def f():
                        mark(f"A{n}.qk{m}")
                        pq = psF.tile([P, NQ], F32, tag=f"f{m}", name="pq")
                        for k in range(KO):
                            nc.tensor.matmul(
                                pq[:], wqk_sb[:, k, m * P:(m + 1) * P],
                                st["get"](k), start=(k == 0), stop=(k == KO - 1))
                        dst = qT_sb if m == 0 else kT_sb
                        nc.vector.tensor_scalar_add(
                            dst[:, n * NQ:(n + 1) * NQ], pq[:],
                            bqk_sb[:, m:m + 1])
                    return f

                def q_vT():
                    # vT[feat, tok] accumulated with wv stationary, staged to
                    # SBUF for the PE transpose back to natural layout
                    mark(f"A{n}.vT")
                    pvT = psF.tile([P, NQ], F32, tag="f0", name="pvT")
                    for k in range(KO):
                        nc.tensor.matmul(
                            pvT[:], wv_sb[:, k, :], st["get"](k),
                            start=(k == 0), stop=(k == KO - 1))
                    vt = ysp.tile([P, NQ], BF16, tag="vt", name="vt", bufs=4)
                    nc.vector.tensor_copy(vt[:], pvT[:])
                    st["vt"] = vt

                def q_tp():
                    # PE-transpose vT back to natural [token, feat] layout
                    # for the PV stationary operand — all four [128,128]
                    # blocks into one PSUM tile, one strided copy per head
                    def f():
                        mark(f"A{n}.tp")
                        tp = psF.tile([P, 4, P], BF16, tag="f0", name="tp")
                        for m2 in range(4):
                            nc.tensor.transpose(
                                tp[:, m2, :],
                                st["vt"][:, m2 * P:(m2 + 1) * P], ident_sb[:])
                        for h in range(2):
                            nc.vector.tensor_copy(
                                v_sb[h][:, n * 4:(n + 1) * 4, 0:D],
                                tp[:, :, h * D:(h + 1) * D])
                    return f

                return q_dma, [q_qk(0), q_qk(1), q_vT, q_tp()]

            def make_C_quantum(m):
                def f():
                    mark(f"C.m{m}")
                    ys = ysp.tile([P, C], BF16, name="ys", tag="ys", bufs=24)
                    if drain_box[0]:
                        # drain: rotate over all free PSUM banks (psF pair,
                        # psO pair, psS 2-bank tiles) so back-to-back quanta
                        # never wait on a bank recycling through a copy
                        variant = (ccount_box[0] // 2) % 3
                    else:
                        variant = 0
                    pys = []
                    if variant == 2:
                        s2 = psS_g.tile([P, 2 * NQ], F32, tag="s", name="py2")
                        pys = [s2[:, 0:NQ], s2[:, NQ:2 * NQ]]
                    else:
                        pool, pfx = cpool_box[0] if variant == 0 else (psO_g, "o")
                        pys = [pool.tile([P, NQ], F32, tag=f"{pfx}{n2}",
                                         name="py")[:] for n2 in range(2)]
                    for n2 in range(2):
                        py = pys[n2]
                        nc.tensor.matmul(
                            py, attns_sb[:, m * P:(m + 1) * P],
                            wproj_sb[:, n2 * NQ:(n2 + 1) * NQ],
                            start=True, stop=True)
                        if alt_box[0] and ccount_box[0] % 2 == 0:
                            nc.scalar.copy(ys[:, n2 * NQ:(n2 + 1) * NQ], py)
                        else:
                            nc.vector.tensor_copy(
                                ys[:, n2 * NQ:(n2 + 1) * NQ], py)
                        ccount_box[0] += 1
                        if m == 31:
                            # very last token block: ship each half as its
                            # copy lands, on separate issue queues so
                            # neither blocks behind the other's data wait
                            eng = nc.sync if n2 == 0 else nc.scalar
                            eng.dma_start(
                                y_d[m * P:(m + 1) * P,
                                    n2 * NQ:(n2 + 1) * NQ],
                                ys[:, n2 * NQ:(n2 + 1) * NQ])
                    if m != 31:
                        if drain_box[0]:
                            # drain stores on the two HWDGE issue queues —
                            # the SWDGE path would serialize 1us descriptor
                            # preps on the Pool engine right at the finish
                            eng = nc.scalar if m % 2 == 0 else nc.sync
                        else:
                            # mid-kernel: alternate the HWDGE (sync) and
                            # SWDGE (gpsimd) paths so they don't serialize
                            # on one DGE
                            eng = nc.sync if m % 2 == 0 else nc.gpsimd
                        eng.dma_start(y_d[m * P:(m + 1) * P, :], ys[:])
                return f

            # ---- stage B chunk: attention for batch b, q-chunk i ----
            js_left_box = [80]  # total j-steps over all B chunks

            s_tiles = {}   # (b, i, j) -> pending score tile

            def emit_s(b, i, j):
                # diagonal superblock: columns < dlt*KB are fully masked
                # and skipped in S, exp and PV alike
                nq0 = b * T + i * NQ
                lo = max(j - 4 * i, 0) * KB
                s = psS_g.tile([P, 2 * NQ], F32, tag="s", name="s")
                for h in range(2):
                    c0 = h * NQ
                    nc.tensor.matmul(
                        s[:, c0 + lo:c0 + NQ],
                        kT_sb[h * D:(h + 1) * D,
                              b * T + j * KB: b * T + (j + 1) * KB],
                        qT_sb[h * D:(h + 1) * D, nq0 + lo:nq0 + NQ],
                        start=True, stop=True)
                s_tiles[(b, i, j)] = s

            def emit_B(b, i, nxt=None, burst=None):
                nq0 = b * T + i * NQ
                jmax = 4 * i + 4
                po = [psO_g.tile([P, NQ], F32, tag=f"o{h}", name=f"po{h}")
                      for h in range(2)]

                if (b, i, 0) not in s_tiles:
                    emit_s(b, i, 0)
                if jmax > 1:
                    # S(1) before the boundary A-burst so exp(0)/exp(1) both
                    # run under it and PV(0)/PV(1) never wait at a boundary
                    emit_s(b, i, 1)
                if burst is not None:
                    burst()
                pump(2, max_c=1)
                budget0 = (len(fillA) + len(fillC)) * jmax // js_left_box[0]
                js_left_box[0] -= jmax
                taken = 0
                for j in range(jmax):
                    mark(f"B{b}.{i}.j{j}")
                    step_box[0] += 1
                    if j + 1 < jmax:
                        if (b, i, j + 1) not in s_tiles:
                            emit_s(b, i, j + 1)
                    elif nxt is not None:
                        # pre-emit the next chunk's first score block so its
                        # exp latency hides under this chunk's tail and the
                        # boundary A-quanta burst
                        emit_s(nxt[0], nxt[1], 0)
                    pt = ptp.tile([P, 2 * NQ], BF16, name="pt")
                    s = s_tiles.pop((b, i, j))
                    dlt = j - 4 * i
                    lo = max(dlt, 0) * KB
                    if dlt >= 0:
                        # one exp over both heads' unmasked ranges via a
                        # strided AP (block NQ-lo, stride NQ) — halves the
                        # per-instruction PSUM-access overhead
                        s_v = s[:].rearrange("p (g q) -> p g q", g=2)
                        pt_v = pt[:].rearrange("p (g q) -> p g q", g=2)
                        nc.scalar.activation(
                            pt_v[:, :, lo:NQ], s_v[:, :, lo:NQ],
                            mybir.ActivationFunctionType.Exp)
                        pt_t = pt_v[:, :, lo:lo + KB]
                        nc.vector.tensor_mul(
                            pt_t, pt_t,
                            masks_sb[:].unsqueeze(1).broadcast_to(
                                (P, 2, KB)))
                    else:
                        nc.scalar.activation(
                            pt[:], s[:],
                            mybir.ActivationFunctionType.Exp)
                    want = budget0 * (j + 1) // jmax
                    if want > taken:
                        pump(want - taken, max_c=1)
                        taken = want
                    for h in range(2):
                        nc.tensor.matmul(
                            po[h][:, lo:NQ], v_sb[h][:, b * KBB + j, :],
                            pt[:, h * NQ + lo:(h + 1) * NQ],
                            start=(j == 0), stop=(j == jmax - 1))
                        if j == jmax - 1:
                            # normalize this head immediately: its recip
                            # runs on DVE while PE starts the other head.
                            # On the final chunk the multiply goes in column
                            # halves so the first drain projections only
                            # wait on their own half
                            rc = ptp.tile([D, NQ], F32, tag="rc", name="rc")
                            nc.vector.reciprocal(rc[:], po[h][D:2 * D, :])
                            nsp = 2 if (b, i) == (1, 3) else 1
                            for q2 in range(nsp):
                                c2 = q2 * (NQ // nsp)
                                nc.vector.tensor_mul(
                                    attns_sb[h * D:(h + 1) * D,
                                             nq0 + c2:nq0 + c2 + NQ // nsp],
                                    po[h][0:D, c2:c2 + NQ // nsp],
                                    rc[:, c2:c2 + NQ // nsp])

            # ---- interleaved emission ----
            with tc.tile_pool(name="psF", bufs=1, space="PSUM") as psF:
                cpool_box[0] = (psF, "f")
                with tc.tile_pool(name="psS", bufs=2, space="PSUM") as psS_g, \
                     tc.tile_pool(name="psO", bufs=1, space="PSUM") as psO_g:
                    # chunk DMAs lead their compute quanta by 2 chunks so
                    # the serial DMA queue stays ahead of the PE; transpose
                    # quanta trail one chunk so their vt copy (DVE) is
                    # long done when the PE reaches them
                    dmas, comps = zip(*(make_A_quanta(n, psF)
                                        for n in range(NCHUNK)))
                    fillA.append(dmas[0])
                    fillA.append(dmas[1])
                    last_idx = {}
                    for n in range(NCHUNK):
                        for q in comps[n][:2]:
                            fillA.append(q)
                        if n == NCHUNK - 1:
                            # the last B chunk only needs its own q/k before
                            # starting; its vT/tp quanta serve as late
                            # fillers for its exp-bound j-steps
                            last_idx[n] = len(fillA)
                        fillA.append(comps[n][2])
                        if n >= 1:
                            for q in comps[n - 1][3:]:
                                fillA.append(q)
                            last_idx[n - 1] = len(fillA)
                        if n + 2 < NCHUNK:
                            fillA.append(dmas[n + 2])
                    for q in comps[NCHUNK - 1][3:]:
                        fillA.append(q)
                    a_total = len(fillA)

                    # PE warm-up: throwaway matmuls on a memset tile keep the
                    # tensor engine's continuous-execution run (p-state ramp)
                    # alive while the first input DMAs are in flight — the
                    # first real matmuls then run at full clock.
                    for w in range(7):
                        wps = psF.tile([P, NQ], F32, tag="f0", name="warm")
                        nc.tensor.matmul(wps[:], warm_sb[:, 0:P], warm_sb[:],
                                         start=True, stop=True)

                    # prologue, ordered by first use: the opening matmul
                    # needs only wqk subtile 0 + the first x half; the rest
                    # interleaves so no qk matmul ever waits more than one
                    # transfer. Small residents go on the Act queue so their
                    # HWDGE slots interleave with the xt loads.
                    xt0 = xtp.tile([P, KO, NQ], BF16, name="xtc", tag="xtc",
                                   bufs=4)
                    xt0_box[0] = xt0
                    nc.sync.dma_start(wqk_sb[:, 0:1, :], wqk_dr[:, 0:1, :])
                    nc.sync.dma_start(xt0[:, 0:4, :], xt_dr[:, 0:4, 0:NQ])
                    nc.sync.dma_start(wqk_sb[:, 1:KO, :], wqk_dr[:, 1:KO, :])
                    nc.sync.dma_start(xt0[:, 4:KO, :], xt_dr[:, 4:KO, 0:NQ])
                    pump(1)                              # binds chunk-0 tile
                    nc.gpsimd.dma_start(bqk_sb[:], bqk_d[:])
                    nc.gpsimd.dma_start(wv_sb[:],
                                        wv_d.rearrange("(ko p) m -> p ko m", p=P))
                    nc.gpsimd.dma_start(ident_sb[:], ident_d[:])
                    nc.gpsimd.dma_start(masks_sb[:], masks_d[:])
                    nc.gpsimd.dma_start(wproj_sb[:], wproj_d[:])
                    for h in range(2):
                        nc.gpsimd.memset(v_sb[h][:, :, D:2 * D], 1.0)
                    pump(8)

                    # longest b1 chunk last: its 16 j-steps pump down the
                    # C backlog so the post-attention drain stays short
                    sched = [(0, i) for i in range(QC)] + \
                            [(1, i) for i in [0, 1, 2, 3]]
                    for idx, (b, i) in enumerate(sched):
                        if True:
                            nxt = sched[idx + 1] if idx + 1 < len(sched) else None
                            if idx == len(sched) - 1:
                                alt_box[0] = True
                            done = last_idx[b * QC + i]

                            def burst(done=done):
                                # A chunks needed by this B chunk first
                                while a_total - len(fillA) < done:
                                    pump(1)
                            emit_B(b, i, nxt, burst)
                            # this chunk's slice of the projection is final
                            for m in range(4 * i + 16 * b, 4 * i + 16 * b + 4):
                                fillC.append((step_box[0], make_C_quantum(m)))
                    # trailing drain (still inside the attention PSUM
                    # scopes: a fresh pool here would emit an all-queue
                    # barrier that stalls PE and resets its p-state)
                    drain_box[0] = True
                    while fillA or fillC:
                        pump(1)

    nc.compile()
    return nc


def _host_prep(x, W_qkv, b_qkv, W_proj, b_proj):
    x = np.ascontiguousarray(np.asarray(x, dtype=np.float32))
    W_qkv = np.asarray(W_qkv, dtype=np.float32)
    b_qkv = np.asarray(b_qkv, dtype=np.float32)
    W_proj = np.asarray(W_proj, dtype=np.float32)
    b_proj = np.asarray(b_proj, dtype=np.float32)

    xT = np.ascontiguousarray(x.reshape(TOK, C).T.astype(BFNP))  # [1024, 4096]
    scale = np.float32(1.0 / np.sqrt(D))

    masks = np.ascontiguousarray(
        np.triu(np.ones((P, P), dtype=np.float32)).astype(BFNP))  # [tk, tq]
    ident = np.ascontiguousarray(np.eye(P, dtype=np.float32).astype(BFNP))

    in_maps = []
    for c in range(8):
        s0, s1 = c * P, (c + 1) * P
        wq = W_qkv[:, s0:s1] * scale
        wk = W_qkv[:, C + s0:C + s1]
        wv = W_qkv[:, 2 * C + s0:2 * C + s1]
        bq = b_qkv[s0:s1] * scale
        bk = b_qkv[C + s0:C + s1]
        in_maps.append({
            "xt": xT,
            "wqk": np.ascontiguousarray(
                np.concatenate([wq, wk], axis=1).astype(BFNP)),
            "bqk": np.ascontiguousarray(np.stack([bq, bk], axis=1)),
            "wv": np.ascontiguousarray(wv.astype(BFNP)),
            "wproj": np.ascontiguousarray(W_proj[s0:s1, :].astype(BFNP)),
            "masks": masks,
            "ident": ident,
        })
    # constant bias terms folded on host:
    #   out_proj bias + (v-bias row) @ W_proj  (v bias passes through softmax)
    ybias = b_qkv[2 * C:3 * C] @ W_proj + b_proj  # [1024]
    return in_maps, ybias


def kernel(x, W_qkv, b_qkv, W_proj, b_proj):
    if "nc" not in _CACHE:
        _CACHE["nc"] = _build()
    nc = _CACHE["nc"]
    in_maps, ybias = _host_prep(x, W_qkv, b_qkv, W_proj, b_proj)
    try:
        res = run_bass_kernel_spmd(nc, in_maps, core_ids=list(range(8)))
    except Exception:
        # transient device errors (NRT_EXEC_UNIT_UNRECOVERABLE) heal on retry
        res = run_bass_kernel_spmd(nc, in_maps, core_ids=list(range(8)))
    y = np.zeros((TOK, C), dtype=np.float32)
    for c in range(8):
        y += np.asarray(res.results[c]["y"], dtype=np.float32)
    y += ybias[None, :].astype(np.float32)
    return y.reshape(B, T, C)
